# revision 15
# baseline (speedup 1.0000x reference)
"""BiLSTM classifier kernel for Trainium2 (8 NeuronCores, data-parallel).

Model: emb lookup -> 2-layer bidirectional LSTM (H=128) -> FC head.
Sharding: batch (128) split 8 ways; weights replicated; no collectives.

Per-core plan:
  - Host dedups this core's token ids (<= BL*T unique) so indices fit int16,
    then dma_gather(transpose=True) lands the embedding pre-transposed
    [E(pad 128) x BL*T] in SBUF as bf16.
  - Input projections (xp = W_ih @ xe + b) are computed just-in-time per
    block with big matmuls; the LSTM recurrence runs fully unrolled with
    weights-stationary [128,128] bf16 gate-chunk matmuls, xp injected into
    the same PSUM accumulation via an identity matmul.
  - Sigmoid-only gate math (tanh(x) = 2*sigmoid(2x)-1, with the 2x folded
    into the g-gate weights) keeps a single ACT table set loaded.
  - Forward/backward directions run as independent interleaved chains;
    layer-0 h sequences stay SBUF-resident (bf16) and feed layer 1 directly.
"""

import os
import sys

import numpy as np

for _p in ("/opt/trn_rl_repo",):
    if os.path.isdir(_p) and _p not in sys.path:
        sys.path.insert(0, _p)

import ml_dtypes

import concourse.bass as bass
import concourse.mybir as mybir
import concourse.tile as tile
from concourse import bacc
from concourse.bass_utils import run_bass_kernel_spmd

V, E, H, B, T, C = 50000, 100, 128, 128, 1024, 2
NCORES = 8
BL = B // NCORES            # 16 batch rows per core
G4 = 4 * BL                 # gate-block width per step (4 gates x BL)

f32 = mybir.dt.float32
bf16 = mybir.dt.bfloat16
i16 = mybir.dt.int16
i32 = mybir.dt.int32
SIG = mybir.ActivationFunctionType.Sigmoid
IDENT = mybir.ActivationFunctionType.Identity
MULT = mybir.AluOpType.mult
ADD = mybir.AluOpType.add
SUB = mybir.AluOpType.subtract

bf = ml_dtypes.bfloat16


# ----------------------------------------------------------------------------
# host-side weight prep
# ----------------------------------------------------------------------------

def _prep_dir(w_ih, w_hh, b_ih, b_hh, h_half_in=False):
    """Gate reorder (i,f,g,o)->(i,f,o,g), scale g rows by 2 (tanh via
    sigmoid), and scale w_hh (and w_ih when its input is h'=h/2) by 2."""
    perm = np.concatenate([
        np.arange(0, H), np.arange(H, 2 * H),
        np.arange(3 * H, 4 * H), np.arange(2 * H, 3 * H)])
    w_ih_p = np.asarray(w_ih, np.float32)[perm].copy()
    w_hh_p = np.asarray(w_hh, np.float32)[perm].copy()
    b_p = (np.asarray(b_ih, np.float32) + np.asarray(b_hh, np.float32))[perm].copy()
    w_ih_p[3 * H:] *= 2.0
    w_hh_p[3 * H:] *= 2.0
    b_p[3 * H:] *= 2.0
    w_hh_p *= 2.0                 # recurrent input is h' = h/2
    if h_half_in:
        w_ih_p *= 2.0             # layer-1 input is concat(h'/..) too
    return w_ih_p, w_hh_p, b_p


def _rec_lhsT(w_hh_p):
    """[4H,H] -> [4,128,128] stationary chunks (lhsT = chunk.T)."""
    return np.stack([w_hh_p[g * H:(g + 1) * H].T for g in range(4)]).astype(bf)


def _proj0_lhsT(w_ih_p):
    """[4H,E] -> [4,128,128] (K padded E->128)."""
    out = np.zeros((4, 128, 128), np.float32)
    for g in range(4):
        out[g, :E, :] = w_ih_p[g * H:(g + 1) * H].T
    return out.astype(bf)


def _proj1_lhsT(w_ih_p):
    """[4H,2H] -> [4,2,128,128] (k-tile 0 = fwd half, 1 = bwd half)."""
    out = np.zeros((4, 2, 128, 128), np.float32)
    for g in range(4):
        for k in range(2):
            out[g, k] = w_ih_p[g * H:(g + 1) * H, k * H:(k + 1) * H].T
    return out.astype(bf)


def prep_shared(inp):
    """Weight tensors shared by all cores."""
    d = {}
    rec = np.zeros((2, 2, 4, 128, 128), bf)
    bias = np.zeros((2, 2, 128, 4), np.float32)
    p0 = np.zeros((2, 4, 128, 128), bf)
    p1 = np.zeros((2, 4, 2, 128, 128), bf)
    for l in (0, 1):
        for di, sfx in enumerate(("", "_rev")):
            w_ih_p, w_hh_p, b_p = _prep_dir(
                inp[f"w_ih_l{l}{sfx}"], inp[f"w_hh_l{l}{sfx}"],
                inp[f"b_ih_l{l}{sfx}"], inp[f"b_hh_l{l}{sfx}"],
                h_half_in=(l == 1))
            rec[l, di] = _rec_lhsT(w_hh_p)
            bias[l, di] = b_p.reshape(4, 128).T
            if l == 0:
                p0[di] = _proj0_lhsT(w_ih_p)
            else:
                p1[di] = _proj1_lhsT(w_ih_p)
    # device layout: straight [K, cols] copies
    d["w_rec"] = np.ascontiguousarray(rec.transpose(0, 1, 3, 2, 4).reshape(2, 2, 128, 512))
    d["bias"] = bias
    d["wp0"] = np.ascontiguousarray(p0.transpose(0, 2, 1, 3).reshape(2, 128, 512))
    d["wp1"] = np.ascontiguousarray(p1.transpose(0, 3, 1, 2, 4).reshape(2, 128, 1024))
    fc_w = np.asarray(inp["fc_w"], np.float32) * 2.0     # [C, 2H]; input h'
    fcw = np.zeros((128, 2, C), np.float32)
    fcw[:, 0] = fc_w[:, :H].T
    fcw[:, 1] = fc_w[:, H:].T
    d["fcw"] = fcw.reshape(128, 2 * C).astype(bf)
    d["fcb"] = np.asarray(inp["fc_b"], np.float32).reshape(C, 1)
    d["ident"] = np.eye(128, dtype=np.float32).astype(bf)
    return d


def prep_core(x_core, emb_bf, Tn=T):
    """Per-core token dedup + int16 index wrap. x_core: [BL, Tn] int."""
    ntok = BL * Tn
    flat = np.asarray(x_core, np.int64).T.reshape(-1)      # n = t*BL + b
    uniq, inv = np.unique(flat, return_inverse=True)
    assert len(uniq) <= ntok
    tab = np.zeros((max(ntok, 128), 128), bf)
    tab[:len(uniq), :E] = emb_bf[uniq]
    # idx[p, ci] = inv[ci*128 + p]  (one gather tile = 128 consecutive tokens)
    idx_full = np.ascontiguousarray(inv.astype(np.int32).reshape(ntok // 128, 128).T)
    return {"emb_tab": tab, "idx": idx_full}


# ----------------------------------------------------------------------------
# device kernel builder
# ----------------------------------------------------------------------------

def build_kernel(Tn=T, TB=64, repeat=0):
    assert Tn % TB == 0
    NTOK = BL * Tn
    # Bacc: auto-inserts gpsimd library reloads and lowers custom
    # instructions for walrus.
    nc = bacc.Bacc()

    emb_tab = nc.dram_tensor("emb_tab", [max(NTOK, 128), 128], bf16,
                             kind="ExternalInput")
    idx_d = nc.dram_tensor("idx", [128, NTOK // 128], i32, kind="ExternalInput")
    wrec_d = nc.dram_tensor("w_rec", [2, 2, 128, 512], bf16,
                            kind="ExternalInput")
    bias_d = nc.dram_tensor("bias", [2, 2, 128, 4], f32, kind="ExternalInput")
    wp0_d = nc.dram_tensor("wp0", [2, 128, 512], bf16, kind="ExternalInput")
    wp1_d = nc.dram_tensor("wp1", [2, 128, 1024], bf16, kind="ExternalInput")
    fcw_d = nc.dram_tensor("fcw", [128, 2 * C], bf16, kind="ExternalInput")
    fcb_d = nc.dram_tensor("fcb", [C, 1], f32, kind="ExternalInput")
    ident_d = nc.dram_tensor("ident", [128, 128], bf16, kind="ExternalInput")
    out_d = nc.dram_tensor("out", [C, BL], f32, kind="ExternalOutput")

    with tile.TileContext(nc) as tc:
        if repeat:
            # timing mode: run the whole computation `repeat` times so the
            # per-iteration time can be extracted by differencing.
            with tc.For_i(0, repeat, 1):
                _build_body(nc, tc, Tn, TB, NTOK,
                            emb_tab, idx_d, wrec_d, bias_d, wp0_d, wp1_d,
                            fcw_d, fcb_d, ident_d, out_d)
        else:
            _build_body(nc, tc, Tn, TB, NTOK,
                        emb_tab, idx_d, wrec_d, bias_d, wp0_d, wp1_d,
                        fcw_d, fcb_d, ident_d, out_d)
    if not nc.is_finalized():
        nc.finalize()
    return nc


def _build_body(nc, tc, Tn, TB, NTOK,
                emb_tab, idx_d, wrec_d, bias_d, wp0_d, wp1_d,
                fcw_d, fcb_d, ident_d, out_d):
    from contextlib import ExitStack
    NBLK = Tn // TB
    ctx = ExitStack()
    persist = ctx.enter_context(tc.tile_pool(name="persist", bufs=1))
    xpp = ctx.enter_context(tc.tile_pool(name="xpw", bufs=2))
    spool = ctx.enter_context(tc.tile_pool(name="spool", bufs=3))
    hpool = ctx.enter_context(tc.tile_pool(name="hpool", bufs=3))
    psum_g = ctx.enter_context(tc.tile_pool(name="psg", bufs=2, space="PSUM"))
    psum_p = ctx.enter_context(tc.tile_pool(name="psp", bufs=2, space="PSUM"))
    psum_tr = ctx.enter_context(tc.tile_pool(name="pstr", bufs=2, space="PSUM"))

    # ---- persistent tiles (separate per direction to avoid false deps)
    xe = persist.tile([128, NTOK], bf16, tag="xe", name="xe")
    seq = [persist.tile([128, Tn, BL], bf16, tag=f"seq{di}", name=f"seq{di}") for di in (0, 1)]
    cst = [persist.tile([128, BL], f32, tag=f"cst{di}", name=f"cst{di}") for di in (0, 1)]
    zh = persist.tile([128, BL], bf16, tag="zh", name="zh")           # zero h_init
    idx_t = persist.tile([128, NTOK // 128], i32, tag="idx", name="idx_t")
    wrec_t = {}
    bias_t = {}
    for l in (0, 1):
        for di in (0, 1):
            wrec_t[(l, di)] = persist.tile([128, 512], bf16, tag=f"wrec{l}{di}", name=f"wrec{l}{di}")
            bias_t[(l, di)] = persist.tile([128, 4], f32, tag=f"bias{l}{di}", name=f"biast{l}{di}")
    wp0_t = [persist.tile([128, 512], bf16, tag=f"wp0{di}", name=f"wp0t{di}") for di in (0, 1)]
    wp1_t = [persist.tile([128, 1024], bf16, tag=f"wp1{di}", name=f"wp1t{di}") for di in (0, 1)]
    fcw_t = persist.tile([128, 2 * C], bf16, tag="fcw", name="fcwt")
    fcb_t = persist.tile([C, 1], f32, tag="fcb", name="fcbt")
    ident_t = persist.tile([128, 128], bf16, tag="ident", name="identt")

    # ---- constant loads
    nc.sync.dma_start(idx_t[:], idx_d[:])
    for l in (0, 1):
        for di in (0, 1):
            nc.sync.dma_start(wrec_t[(l, di)][:], wrec_d[l, di])
            nc.sync.dma_start(bias_t[(l, di)][:], bias_d[l, di])
    for di in (0, 1):
        nc.sync.dma_start(wp0_t[di][:], wp0_d[di])
        nc.sync.dma_start(wp1_t[di][:], wp1_d[di])
    nc.sync.dma_start(fcw_t[:], fcw_d[:])
    nc.sync.dma_start(fcb_t[:], fcb_d[:])
    nc.sync.dma_start(ident_t[:], ident_d[:])
    nc.gpsimd.memset(zh[:], 0.0)
    for di in (0, 1):
        nc.gpsimd.memset(cst[di][:], 0.0)

    # ---- embedding gather: row-gather 128 tokens/call, then PE-transpose so
    # the feature dim lands on partitions: xe[:, n] = emb_tab[idx[n], :].T
    # Gathers are ordered from both sequence ends so the first fwd AND bwd
    # recurrence blocks become ready early.
    NG = NTOK // 128
    order = []
    for k in range((NG + 1) // 2):
        order.append(k)
        if NG - 1 - k != k:
            order.append(NG - 1 - k)
    for ci in order:
        gt = spool.tile([128, 128], bf16, tag="gath", name="gath")
        tr = psum_tr.tile([128, 128], bf16, tag="tr", name="tr")
        nc.gpsimd.indirect_dma_start(
            out=gt[:],
            out_offset=None,
            in_=emb_tab[:],
            in_offset=bass.IndirectOffsetOnAxis(ap=idx_t[:, ci:ci + 1], axis=0),
        )
        nc.tensor.transpose(out=tr[:], in_=gt[:], identity=ident_t[:])
        nc.vector.tensor_copy(out=xe[:, ci * 128:(ci + 1) * 128], in_=tr[:])

    def proj_block(layer, di, blk):
        """xp for TB steps of (layer, dir) -> window tile [128, TB, G4],
        where window[:, tl, g*BL+b] = preact(gate g, unit=partition, b)."""
        w = xpp.tile([128, TB, G4], bf16, tag=f"xpw{di}", name=f"xpw{di}")
        wsrc = wp0_t[di] if layer == 0 else wp1_t[di]
        bias = bias_t[(layer, di)]
        TP = 512 // BL                      # timesteps per psum piece (32)
        for g in range(4):
            for p0 in range(0, TB, TP):
                npc = min(TP, TB - p0) * BL
                ps = psum_p.tile([128, TP, BL], f32, tag="proj", name="projps")
                c0 = (blk * TB + p0) * BL
                if layer == 0:
                    nc.tensor.matmul(
                        ps[:, :npc // BL, :],
                        wsrc[:, g * 128:(g + 1) * 128],
                        xe[:, c0:c0 + npc],
                        start=True, stop=True)
                else:
                    for k in range(2):
                        nc.tensor.matmul(
                            ps[:, :npc // BL, :],
                            wsrc[:, (g * 2 + k) * 128:(g * 2 + k + 1) * 128],
                            seq[k][:, blk * TB + p0: blk * TB + p0 + npc // BL, :],
                            start=(k == 0), stop=(k == 1))
                nc.vector.tensor_scalar(
                    out=w[:, p0:p0 + npc // BL, g * BL:(g + 1) * BL],
                    in0=ps[:, :npc // BL, :],
                    scalar1=bias[:, g:g + 1],
                    scalar2=None,
                    op0=ADD)
        return w

    def lstm_step(layer, di, t_loc, xpw, h_prev, h_out):
        """One LSTM step for one direction. h_prev/h_out: APs [128, BL]."""
        gates = psum_g.tile([128, G4], f32, tag=f"g{di}", name=f"gates{di}")
        s = spool.tile([128, 5 * BL], f32, tag=f"s{di}", name=f"s{di}")
        wrec = wrec_t[(layer, di)]
        c_ap = cst[di][:]
        # xp first (independent of h -> off the critical path); each W-chunk
        # matmul closes its own element range with stop=True.
        nc.tensor.matmul(gates[:], ident_t[:], xpw[:, t_loc, :],
                         start=True, stop=False)
        for g in range(4):
            nc.tensor.matmul(gates[:, g * BL:(g + 1) * BL],
                             wrec[:, g * 128:(g + 1) * 128],
                             h_prev,
                             start=False, stop=True, skip_group_check=True)
        # s[0:4BL] = sigmoid(gates)   (i,f,o,g2)
        nc.scalar.activation(s[:, :G4], gates[:], SIG)
        t1 = spool.tile([128, BL], f32, tag=f"p{di}", name=f"t1{di}")
        q = spool.tile([128, BL], f32, tag=f"q{di}", name=f"q{di}")
        si = s[:, 0:BL]
        sf = s[:, BL:2 * BL]
        so = s[:, 2 * BL:3 * BL]
        sg = s[:, 3 * BL:4 * BL]
        sc = s[:, 4 * BL:5 * BL]
        # c = 2*(sg-0.5)*si + sf*c     (= si*tanh(g) + sf*c)
        nc.vector.scalar_tensor_tensor(out=t1[:], in0=sg, scalar=0.5,
                                       in1=si, op0=SUB, op1=MULT)
        nc.vector.tensor_tensor(out=q[:], in0=sf, in1=c_ap, op=MULT)
        nc.vector.scalar_tensor_tensor(out=c_ap, in0=t1[:], scalar=2.0,
                                       in1=q[:], op0=MULT, op1=ADD)
        nc.scalar.activation(sc, c_ap, SIG, scale=2.0)
        # h' = (sigmoid(2c)-0.5)*so = tanh(c)*so/2; consumers' weights are
        # pre-scaled by 2 on the host.
        nc.vector.scalar_tensor_tensor(out=h_out, in0=sc, scalar=0.5,
                                       in1=so, op0=SUB, op1=MULT)

    # =================== phase A: layer 0 ===================
    for blk in range(NBLK):
        bfw = blk               # fwd block index
        bbw = NBLK - 1 - blk    # bwd block index
        xw_f = proj_block(0, 0, bfw)
        xw_b = proj_block(0, 1, bbw)
        for j in range(TB):
            tf = bfw * TB + j
            tb = bbw * TB + (TB - 1 - j)
            hp_f = zh[:] if tf == 0 else seq[0][:, tf - 1, :]
            lstm_step(0, 0, j, xw_f, hp_f, seq[0][:, tf, :])
            hp_b = zh[:] if tb == Tn - 1 else seq[1][:, tb + 1, :]
            lstm_step(0, 1, TB - 1 - j, xw_b, hp_b, seq[1][:, tb, :])

    # reset c state for layer 1
    for di in (0, 1):
        nc.gpsimd.memset(cst[di][:], 0.0)

    # =================== phase B: layer 1 ===================
    h_fin = [None, None]
    for blk in range(NBLK):
        bfw = blk
        bbw = NBLK - 1 - blk
        xw_f = proj_block(1, 0, bfw)
        xw_b = proj_block(1, 1, bbw)
        for j in range(TB):
            nh_f = hpool.tile([128, BL], bf16, tag="hf", name="hf")
            nh_b = hpool.tile([128, BL], bf16, tag="hb", name="hb")
            lstm_step(1, 0, j, xw_f,
                      zh[:] if h_fin[0] is None else h_fin[0][:], nh_f[:])
            lstm_step(1, 1, TB - 1 - j, xw_b,
                      zh[:] if h_fin[1] is None else h_fin[1][:], nh_b[:])
            h_fin = [nh_f, nh_b]

    # =================== FC head ===================
    fc_ps = psum_p.tile([C, BL], f32, tag="proj", name="fcps")
    nc.tensor.matmul(fc_ps[:], fcw_t[:, 0:C], h_fin[0][:],
                     start=True, stop=False)
    nc.tensor.matmul(fc_ps[:], fcw_t[:, C:2 * C], h_fin[1][:], start=False,
                     stop=True)
    out_sb = persist.tile([C, BL], f32, tag="out", name="outsb")
    nc.scalar.activation(out_sb[:], fc_ps[:], IDENT, bias=fcb_t[:, 0:1])
    nc.sync.dma_start(out_d[:], out_sb[:])

    ctx.close()


# ----------------------------------------------------------------------------
# entry point
# ----------------------------------------------------------------------------

_CACHE = {}


def _get_nc(Tn=T):
    if Tn not in _CACHE:
        _CACHE[Tn] = build_kernel(Tn)
    return _CACHE[Tn]


def kernel(**inputs):
    x = np.asarray(inputs["x"])
    emb_bf = np.asarray(inputs["emb"], np.float32).astype(bf)
    shared = prep_shared(inputs)
    in_maps = []
    for ci in range(NCORES):
        m = dict(shared)
        m.update(prep_core(x[ci * BL:(ci + 1) * BL], emb_bf))
        in_maps.append(m)
    nc = _get_nc(T)
    trace = os.environ.get("BILSTM_TRACE", "") == "1"
    res = run_bass_kernel_spmd(nc, in_maps, core_ids=list(range(NCORES)),
                               trace=trace)
    global LAST_RESULT
    LAST_RESULT = res
    out = np.zeros((B, C), np.float32)
    for ci in range(NCORES):
        out[ci * BL:(ci + 1) * BL] = res.results[ci]["out"].T
    return out


LAST_RESULT = None


# revision 20
# speedup vs baseline: 24.4128x; 24.4128x over previous
"""BiLSTM classifier kernel for Trainium2 (8 NeuronCores, data-parallel).

Model: emb lookup -> 2-layer bidirectional LSTM (H=128) -> FC head.
Sharding: batch (128) split 8 ways; weights replicated; no collectives.

Per-core plan:
  - Host dedups this core's token ids (<= BL*T unique) so indices fit int16,
    then dma_gather(transpose=True) lands the embedding pre-transposed
    [E(pad 128) x BL*T] in SBUF as bf16.
  - Input projections (xp = W_ih @ xe + b) are computed just-in-time per
    block with big matmuls; the LSTM recurrence runs fully unrolled with
    weights-stationary [128,128] bf16 gate-chunk matmuls, xp injected into
    the same PSUM accumulation via an identity matmul.
  - Sigmoid-only gate math (tanh(x) = 2*sigmoid(2x)-1, with the 2x folded
    into the g-gate weights) keeps a single ACT table set loaded.
  - Forward/backward directions run as independent interleaved chains;
    layer-0 h sequences stay SBUF-resident (bf16) and feed layer 1 directly.
"""

import os
import sys

import numpy as np

for _p in ("/opt/trn_rl_repo",):
    if os.path.isdir(_p) and _p not in sys.path:
        sys.path.insert(0, _p)

import ml_dtypes

import concourse.bass as bass
import concourse.mybir as mybir
import concourse.tile as tile
from concourse import bacc
from concourse.bass_utils import run_bass_kernel_spmd

V, E, H, B, T, C = 50000, 100, 128, 128, 1024, 2
NCORES = 8
BL = B // NCORES            # 16 batch rows per core
G4 = 4 * BL                 # gate-block width per step (4 gates x BL)

f32 = mybir.dt.float32
bf16 = mybir.dt.bfloat16
i16 = mybir.dt.int16
i32 = mybir.dt.int32
SIG = mybir.ActivationFunctionType.Sigmoid
IDENT = mybir.ActivationFunctionType.Identity
MULT = mybir.AluOpType.mult
ADD = mybir.AluOpType.add
SUB = mybir.AluOpType.subtract

bf = ml_dtypes.bfloat16


# ----------------------------------------------------------------------------
# host-side weight prep
# ----------------------------------------------------------------------------

def _prep_dir(w_ih, w_hh, b_ih, b_hh, h_half_in=False):
    """Gate reorder (i,f,g,o)->(i,f,o,g), scale g rows by 2 (tanh via
    sigmoid), and scale w_hh (and w_ih when its input is h'=h/2) by 2."""
    perm = np.concatenate([
        np.arange(0, H), np.arange(H, 2 * H),
        np.arange(3 * H, 4 * H), np.arange(2 * H, 3 * H)])
    w_ih_p = np.asarray(w_ih, np.float32)[perm].copy()
    w_hh_p = np.asarray(w_hh, np.float32)[perm].copy()
    b_p = (np.asarray(b_ih, np.float32) + np.asarray(b_hh, np.float32))[perm].copy()
    w_ih_p[3 * H:] *= 2.0
    w_hh_p[3 * H:] *= 2.0
    b_p[3 * H:] *= 2.0
    w_hh_p *= 2.0                 # recurrent input is h' = h/2
    if h_half_in:
        w_ih_p *= 2.0             # layer-1 input is concat(h'/..) too
    return w_ih_p, w_hh_p, b_p


def _rec_lhsT(w_hh_p):
    """[4H,H] -> [4,128,128] stationary chunks (lhsT = chunk.T)."""
    return np.stack([w_hh_p[g * H:(g + 1) * H].T for g in range(4)]).astype(bf)


def _proj0_lhsT(w_ih_p):
    """[4H,E] -> [4,128,128] (K padded E->128)."""
    out = np.zeros((4, 128, 128), np.float32)
    for g in range(4):
        out[g, :E, :] = w_ih_p[g * H:(g + 1) * H].T
    return out.astype(bf)


def _proj1_lhsT(w_ih_p):
    """[4H,2H] -> [4,2,128,128] (k-tile 0 = fwd half, 1 = bwd half)."""
    out = np.zeros((4, 2, 128, 128), np.float32)
    for g in range(4):
        for k in range(2):
            out[g, k] = w_ih_p[g * H:(g + 1) * H, k * H:(k + 1) * H].T
    return out.astype(bf)


def prep_shared(inp):
    """Weight tensors shared by all cores."""
    d = {}
    rec = np.zeros((2, 2, 4, 128, 128), bf)
    bias = np.zeros((2, 2, 128, 4), np.float32)
    p0 = np.zeros((2, 4, 128, 128), bf)
    p1 = np.zeros((2, 4, 2, 128, 128), bf)
    for l in (0, 1):
        for di, sfx in enumerate(("", "_rev")):
            w_ih_p, w_hh_p, b_p = _prep_dir(
                inp[f"w_ih_l{l}{sfx}"], inp[f"w_hh_l{l}{sfx}"],
                inp[f"b_ih_l{l}{sfx}"], inp[f"b_hh_l{l}{sfx}"],
                h_half_in=(l == 1))
            rec[l, di] = _rec_lhsT(w_hh_p)
            bias[l, di] = b_p.reshape(4, 128).T
            if l == 0:
                p0[di] = _proj0_lhsT(w_ih_p)
            else:
                p1[di] = _proj1_lhsT(w_ih_p)
    # device layout: straight [K, cols] copies
    d["w_rec"] = np.ascontiguousarray(rec.transpose(0, 1, 3, 2, 4).reshape(2, 2, 128, 512))
    d["bias"] = bias
    d["wp0"] = np.ascontiguousarray(p0.transpose(0, 2, 1, 3).reshape(2, 128, 512))
    d["wp1"] = np.ascontiguousarray(p1.transpose(0, 3, 1, 2, 4).reshape(2, 128, 1024))
    fc_w = np.asarray(inp["fc_w"], np.float32) * 2.0     # [C, 2H]; input h'
    fcw = np.zeros((128, 2, C), np.float32)
    fcw[:, 0] = fc_w[:, :H].T
    fcw[:, 1] = fc_w[:, H:].T
    d["fcw"] = fcw.reshape(128, 2 * C).astype(bf)
    d["fcb"] = np.asarray(inp["fc_b"], np.float32).reshape(C, 1)
    d["ident"] = np.eye(128, dtype=np.float32).astype(bf)
    return d


def prep_core(x_core, emb_bf, Tn=T):
    """Per-core token dedup + int16 index wrap. x_core: [BL, Tn] int."""
    ntok = BL * Tn
    flat = np.asarray(x_core, np.int64).T.reshape(-1)      # n = t*BL + b
    uniq, inv = np.unique(flat, return_inverse=True)
    assert len(uniq) <= ntok
    tab = np.zeros((max(ntok, 128), 128), bf)
    tab[:len(uniq), :E] = emb_bf[uniq]
    # idx[p, ci] = inv[ci*128 + p]  (one gather tile = 128 consecutive tokens)
    idx_full = np.ascontiguousarray(inv.astype(np.int32).reshape(ntok // 128, 128).T)
    return {"emb_tab": tab, "idx": idx_full}


# ----------------------------------------------------------------------------
# device kernel builder
# ----------------------------------------------------------------------------

def build_kernel(Tn=T, TB=64, repeat=0):
    assert Tn % TB == 0
    NTOK = BL * Tn
    # Bacc: auto-inserts gpsimd library reloads and lowers custom
    # instructions for walrus.
    nc = bacc.Bacc()

    emb_tab = nc.dram_tensor("emb_tab", [max(NTOK, 128), 128], bf16,
                             kind="ExternalInput")
    idx_d = nc.dram_tensor("idx", [128, NTOK // 128], i32, kind="ExternalInput")
    wrec_d = nc.dram_tensor("w_rec", [2, 2, 128, 512], bf16,
                            kind="ExternalInput")
    bias_d = nc.dram_tensor("bias", [2, 2, 128, 4], f32, kind="ExternalInput")
    wp0_d = nc.dram_tensor("wp0", [2, 128, 512], bf16, kind="ExternalInput")
    wp1_d = nc.dram_tensor("wp1", [2, 128, 1024], bf16, kind="ExternalInput")
    fcw_d = nc.dram_tensor("fcw", [128, 2 * C], bf16, kind="ExternalInput")
    fcb_d = nc.dram_tensor("fcb", [C, 1], f32, kind="ExternalInput")
    ident_d = nc.dram_tensor("ident", [128, 128], bf16, kind="ExternalInput")
    out_d = nc.dram_tensor("out", [C, BL], f32, kind="ExternalOutput")

    with tile.TileContext(nc) as tc:
        if repeat:
            # timing mode: run the whole computation `repeat` times so the
            # per-iteration time can be extracted by differencing.
            with tc.For_i(0, repeat, 1):
                _build_body(nc, tc, Tn, TB, NTOK,
                            emb_tab, idx_d, wrec_d, bias_d, wp0_d, wp1_d,
                            fcw_d, fcb_d, ident_d, out_d)
        else:
            _build_body(nc, tc, Tn, TB, NTOK,
                        emb_tab, idx_d, wrec_d, bias_d, wp0_d, wp1_d,
                        fcw_d, fcb_d, ident_d, out_d)
    if not nc.is_finalized():
        nc.finalize()
    return nc


def _build_body(nc, tc, Tn, TB, NTOK,
                emb_tab, idx_d, wrec_d, bias_d, wp0_d, wp1_d,
                fcw_d, fcb_d, ident_d, out_d):
    from contextlib import ExitStack
    NBLK = Tn // TB
    ctx = ExitStack()
    persist = ctx.enter_context(tc.tile_pool(name="persist", bufs=1))
    xpp = ctx.enter_context(tc.tile_pool(name="xpw", bufs=2))
    spool = ctx.enter_context(tc.tile_pool(name="spool", bufs=3))
    hpool = ctx.enter_context(tc.tile_pool(name="hpool", bufs=3))
    psum_g = ctx.enter_context(tc.tile_pool(name="psg", bufs=2, space="PSUM"))
    psum_p = ctx.enter_context(tc.tile_pool(name="psp", bufs=2, space="PSUM"))
    psum_tr = ctx.enter_context(tc.tile_pool(name="pstr", bufs=2, space="PSUM"))

    # ---- persistent tiles (separate per direction to avoid false deps)
    xe = persist.tile([128, NTOK], bf16, tag="xe", name="xe")
    seq = [persist.tile([128, Tn, BL], bf16, tag=f"seq{di}", name=f"seq{di}") for di in (0, 1)]
    cst = [persist.tile([128, BL], f32, tag=f"cst{di}", name=f"cst{di}") for di in (0, 1)]
    zh = persist.tile([128, BL], bf16, tag="zh", name="zh")           # zero h_init
    idx_t = persist.tile([128, NTOK // 128], i32, tag="idx", name="idx_t")
    wrec_t = {}
    bias_t = {}
    for l in (0, 1):
        for di in (0, 1):
            wrec_t[(l, di)] = persist.tile([128, 512], bf16, tag=f"wrec{l}{di}", name=f"wrec{l}{di}")
            bias_t[(l, di)] = persist.tile([128, 4], f32, tag=f"bias{l}{di}", name=f"biast{l}{di}")
    wp0_t = [persist.tile([128, 512], bf16, tag=f"wp0{di}", name=f"wp0t{di}") for di in (0, 1)]
    wp1_t = [persist.tile([128, 1024], bf16, tag=f"wp1{di}", name=f"wp1t{di}") for di in (0, 1)]
    fcw_t = persist.tile([128, 2 * C], bf16, tag="fcw", name="fcwt")
    fcb_t = persist.tile([C, 1], f32, tag="fcb", name="fcbt")
    ident_t = persist.tile([128, 128], bf16, tag="ident", name="identt")

    # ---- constant loads
    nc.sync.dma_start(idx_t[:], idx_d[:])
    for l in (0, 1):
        for di in (0, 1):
            nc.sync.dma_start(wrec_t[(l, di)][:], wrec_d[l, di])
            nc.sync.dma_start(bias_t[(l, di)][:], bias_d[l, di])
    for di in (0, 1):
        nc.sync.dma_start(wp0_t[di][:], wp0_d[di])
        nc.sync.dma_start(wp1_t[di][:], wp1_d[di])
    nc.sync.dma_start(fcw_t[:], fcw_d[:])
    nc.sync.dma_start(fcb_t[:], fcb_d[:])
    nc.sync.dma_start(ident_t[:], ident_d[:])
    nc.gpsimd.memset(zh[:], 0.0)
    for di in (0, 1):
        nc.gpsimd.memset(cst[di][:], 0.0)

    # ---- embedding gather: row-gather 128 tokens/call, then PE-transpose so
    # the feature dim lands on partitions: xe[:, n] = emb_tab[idx[n], :].T
    # Gathers are ordered from both sequence ends so the first fwd AND bwd
    # recurrence blocks become ready early.
    NG = NTOK // 128
    order = []
    for k in range((NG + 1) // 2):
        order.append(k)
        if NG - 1 - k != k:
            order.append(NG - 1 - k)
    for ci in order:
        gt = spool.tile([128, 128], bf16, tag="gath", name="gath")
        tr = psum_tr.tile([128, 128], bf16, tag="tr", name="tr")
        nc.gpsimd.indirect_dma_start(
            out=gt[:],
            out_offset=None,
            in_=emb_tab[:],
            in_offset=bass.IndirectOffsetOnAxis(ap=idx_t[:, ci:ci + 1], axis=0),
        )
        nc.tensor.transpose(out=tr[:], in_=gt[:], identity=ident_t[:])
        nc.vector.tensor_copy(out=xe[:, ci * 128:(ci + 1) * 128], in_=tr[:])

    def proj_block(layer, di, blk):
        """xp for TB steps of (layer, dir) -> window tile [128, TB, G4],
        where window[:, tl, g*BL+b] = preact(gate g, unit=partition, b)."""
        w = xpp.tile([128, TB, G4], bf16, tag=f"xpw{di}", name=f"xpw{di}")
        wsrc = wp0_t[di] if layer == 0 else wp1_t[di]
        bias = bias_t[(layer, di)]
        TP = 512 // BL                      # timesteps per psum piece (32)
        for g in range(4):
            for p0 in range(0, TB, TP):
                npc = min(TP, TB - p0) * BL
                ps = psum_p.tile([128, TP, BL], f32, tag="proj", name="projps")
                c0 = (blk * TB + p0) * BL
                if layer == 0:
                    nc.tensor.matmul(
                        ps[:, :npc // BL, :],
                        wsrc[:, g * 128:(g + 1) * 128],
                        xe[:, c0:c0 + npc],
                        start=True, stop=True)
                else:
                    for k in range(2):
                        nc.tensor.matmul(
                            ps[:, :npc // BL, :],
                            wsrc[:, (g * 2 + k) * 128:(g * 2 + k + 1) * 128],
                            seq[k][:, blk * TB + p0: blk * TB + p0 + npc // BL, :],
                            start=(k == 0), stop=(k == 1))
                nc.vector.tensor_scalar(
                    out=w[:, p0:p0 + npc // BL, g * BL:(g + 1) * BL],
                    in0=ps[:, :npc // BL, :],
                    scalar1=bias[:, g:g + 1],
                    scalar2=None,
                    op0=ADD)
        return w

    def lstm_step_pair(layer, specs):
        """Emit one LSTM step for both directions, op-type interleaved so
        neither chain head-of-line-blocks the other in the strict-FIFO
        engine queues. specs: [(di, t_loc, xpw, h_prev, h_out), ...]."""
        st = {}
        for di, t_loc, xpw, h_prev, h_out in specs:
            gates = psum_g.tile([128, G4], f32, tag=f"g{di}", name=f"gates{di}")
            s = spool.tile([128, 5 * BL], f32, tag=f"s{di}", name=f"s{di}")
            t1 = spool.tile([128, BL], f32, tag=f"p{di}", name=f"t1{di}")
            q = spool.tile([128, BL], f32, tag=f"q{di}", name=f"q{di}")
            st[di] = (gates, s, t1, q)
        for di, t_loc, xpw, h_prev, h_out in specs:
            gates = st[di][0]
            wrec = wrec_t[(layer, di)]
            for g in range(4):
                nc.tensor.matmul(gates[:, g * BL:(g + 1) * BL],
                                 wrec[:, g * 128:(g + 1) * 128],
                                 h_prev,
                                 start=(g == 0), stop=False)
            nc.tensor.matmul(gates[:], ident_t[:], xpw[:, t_loc, :],
                             start=False, stop=True)
        for di, t_loc, xpw, h_prev, h_out in specs:
            gates, s = st[di][0], st[di][1]
            nc.scalar.activation(s[:, :G4], gates[:], SIG)
        for di, t_loc, xpw, h_prev, h_out in specs:
            gates, s, t1, q = st[di]
            c_ap = cst[di][:]
            # c = 2*(sg-0.5)*si + sf*c     (= si*tanh(g) + sf*c)
            nc.vector.scalar_tensor_tensor(out=t1[:], in0=s[:, 3 * BL:4 * BL],
                                           scalar=0.5, in1=s[:, 0:BL],
                                           op0=SUB, op1=MULT)
            nc.vector.tensor_tensor(out=q[:], in0=s[:, BL:2 * BL], in1=c_ap,
                                    op=MULT)
            nc.vector.scalar_tensor_tensor(out=c_ap, in0=t1[:], scalar=2.0,
                                           in1=q[:], op0=MULT, op1=ADD)
        for di, t_loc, xpw, h_prev, h_out in specs:
            s, c_ap = st[di][1], cst[di][:]
            nc.scalar.activation(s[:, 4 * BL:5 * BL], c_ap, SIG, scale=2.0)
        for di, t_loc, xpw, h_prev, h_out in specs:
            s = st[di][1]
            # h' = (sigmoid(2c)-0.5)*so = tanh(c)*so/2; consumers' weights
            # are pre-scaled by 2 on the host.
            nc.vector.scalar_tensor_tensor(out=h_out, in0=s[:, 4 * BL:5 * BL],
                                           scalar=0.5, in1=s[:, 2 * BL:3 * BL],
                                           op0=SUB, op1=MULT)

    # =================== phase A: layer 0 ===================
    for blk in range(NBLK):
        bfw = blk               # fwd block index
        bbw = NBLK - 1 - blk    # bwd block index
        xw_f = proj_block(0, 0, bfw)
        xw_b = proj_block(0, 1, bbw)
        for j in range(TB):
            tf = bfw * TB + j
            tb = bbw * TB + (TB - 1 - j)
            hp_f = zh[:] if tf == 0 else seq[0][:, tf - 1, :]
            hp_b = zh[:] if tb == Tn - 1 else seq[1][:, tb + 1, :]
            lstm_step_pair(0, [
                (0, j, xw_f, hp_f, seq[0][:, tf, :]),
                (1, TB - 1 - j, xw_b, hp_b, seq[1][:, tb, :])])

    # reset c state for layer 1
    for di in (0, 1):
        nc.gpsimd.memset(cst[di][:], 0.0)

    # =================== phase B: layer 1 ===================
    h_fin = [None, None]
    for blk in range(NBLK):
        bfw = blk
        bbw = NBLK - 1 - blk
        xw_f = proj_block(1, 0, bfw)
        xw_b = proj_block(1, 1, bbw)
        for j in range(TB):
            nh_f = hpool.tile([128, BL], bf16, tag="hf", name="hf")
            nh_b = hpool.tile([128, BL], bf16, tag="hb", name="hb")
            lstm_step_pair(1, [
                (0, j, xw_f,
                 zh[:] if h_fin[0] is None else h_fin[0][:], nh_f[:]),
                (1, TB - 1 - j, xw_b,
                 zh[:] if h_fin[1] is None else h_fin[1][:], nh_b[:])])
            h_fin = [nh_f, nh_b]

    # =================== FC head ===================
    fc_ps = psum_p.tile([C, BL], f32, tag="proj", name="fcps")
    nc.tensor.matmul(fc_ps[:], fcw_t[:, 0:C], h_fin[0][:],
                     start=True, stop=False)
    nc.tensor.matmul(fc_ps[:], fcw_t[:, C:2 * C], h_fin[1][:], start=False,
                     stop=True)
    out_sb = persist.tile([C, BL], f32, tag="out", name="outsb")
    nc.scalar.activation(out_sb[:], fc_ps[:], IDENT, bias=fcb_t[:, 0:1])
    nc.sync.dma_start(out_d[:], out_sb[:])

    ctx.close()


# ----------------------------------------------------------------------------
# entry point
# ----------------------------------------------------------------------------

_CACHE = {}


def _get_nc(Tn=T):
    if Tn not in _CACHE:
        _CACHE[Tn] = build_kernel(Tn)
    return _CACHE[Tn]


def kernel(**inputs):
    x = np.asarray(inputs["x"])
    emb_bf = np.asarray(inputs["emb"], np.float32).astype(bf)
    shared = prep_shared(inputs)
    in_maps = []
    for ci in range(NCORES):
        m = dict(shared)
        m.update(prep_core(x[ci * BL:(ci + 1) * BL], emb_bf))
        in_maps.append(m)
    nc = _get_nc(T)
    trace = os.environ.get("BILSTM_TRACE", "") == "1"
    res = run_bass_kernel_spmd(nc, in_maps, core_ids=list(range(NCORES)),
                               trace=trace)
    global LAST_RESULT
    LAST_RESULT = res
    out = np.zeros((B, C), np.float32)
    for ci in range(NCORES):
        out[ci * BL:(ci + 1) * BL] = res.results[ci]["out"].T
    return out


LAST_RESULT = None
